# revision 5
# baseline (speedup 1.0000x reference)
"""CenterLoss Trainium2 kernel.

reference semantics:
    feats  = features.reshape(4096, 96)
    label  = argmax(predicts.reshape(4096, 6625), axis=1)   # first occurrence
    d[n]   = ||feats[n] - centers[label[n]]||^2
    loss   = (sum_n clip(d[n], 1e-12, 1e12) + (4096*6625-4096)*1e-12) / 4096

Sharding: data-parallel over the flattened 4096-row batch axis, 512 rows per
core across 8 cores; centers replicated. Each core returns its 512 per-row
distances; the host does the final (tiny) reduction.

Per-core pipeline, per 128-row tile:
  1. stream predicts row-tile [128, 6625] HBM->SBUF (the memory-bound part)
  2. chunked max-reduce over a [128, 25, 265] view -> cmax [128, 25]
  3. Max8 + MaxIndex on cmax -> winning chunk id per row
  4. indirect-DMA gather of the winning 265-wide chunk from HBM
  5. Max8 + MaxIndex within the chunk -> label = chunk*265 + pos
  6. indirect-DMA gather of centers[label] -> [128, 96]
  7. d = sum((f - c)^2) via vector sub + ACT Square with accum_out
"""

import numpy as np

NUM_CLASSES = 6625
FEAT_DIM = 96
N_ROWS = 4096           # B*T = 64*64
N_CORES = 8
ROWS_PER_CORE = N_ROWS // N_CORES   # 512
P = 128                 # partitions
N_TILES = ROWS_PER_CORE // P        # 4 row-tiles per core
CH = 265                # chunk size (6625 = 25 * 265)
NCHUNK = NUM_CLASSES // CH          # 25

_CACHE = {}


def _build_nc(reps=1):
    if ("nc", reps) in _CACHE:
        return _CACHE[("nc", reps)]

    from contextlib import ExitStack

    import concourse.bass as bass
    import concourse.tile as tile
    from concourse import bacc, mybir

    nc = bacc.Bacc(
        "TRN2",
        target_bir_lowering=False,
        debug=False,
        num_devices=N_CORES,
    )

    predicts = nc.dram_tensor(
        "predicts", [ROWS_PER_CORE, NUM_CLASSES], mybir.dt.float32,
        kind="ExternalInput",
    )
    features = nc.dram_tensor(
        "features", [ROWS_PER_CORE, FEAT_DIM], mybir.dt.float32,
        kind="ExternalInput",
    )
    centers = nc.dram_tensor(
        "centers", [NUM_CLASSES, FEAT_DIM], mybir.dt.float32,
        kind="ExternalInput",
    )
    out = nc.dram_tensor(
        "out", [P, N_TILES], mybir.dt.float32, kind="ExternalOutput",
    )

    fadd = mybir.AluOpType.add
    fmul = mybir.AluOpType.mult

    with tile.TileContext(nc) as tc:
        with ExitStack() as ctx:
            xpool = ctx.enter_context(tc.tile_pool(name="x", bufs=2))
            small = ctx.enter_context(tc.tile_pool(name="small", bufs=3))
            const = ctx.enter_context(tc.tile_pool(name="const", bufs=1))

            # rowbase[p] = p * NCHUNK  (chunk-row id of local row p, tile 0)
            rowbase_i = const.tile([P, 1], mybir.dt.int32)
            nc.gpsimd.iota(
                rowbase_i[:], pattern=[[0, 1]], base=0,
                channel_multiplier=NCHUNK,
            )
            rowbase_f = const.tile([P, 1], mybir.dt.float32)
            nc.vector.tensor_copy(rowbase_f[:], rowbase_i[:])

            # per-row distances, one column per row-tile
            acc = const.tile([P, N_TILES], mybir.dt.float32)

            # predicts viewed as rows of 265 elements: [512*25, 265]
            pred_chunks = predicts.ap().rearrange("r (a b) -> (r a) b", b=CH)

            for t in [t for _ in range(reps) for t in range(N_TILES)]:
                x = xpool.tile([P, NUM_CLASSES], mybir.dt.float32, tag="x")
                # split the big stream DMA so the reduce can start earlier
                h = (NCHUNK // 2) * CH  # 12 chunks
                nc.sync.dma_start(
                    x[:, :h], predicts.ap()[t * P:(t + 1) * P, :h])
                nc.sync.dma_start(
                    x[:, h:], predicts.ap()[t * P:(t + 1) * P, h:])

                f = small.tile([P, FEAT_DIM], mybir.dt.float32, tag="feat")
                nc.sync.dma_start(f[:], features.ap()[t * P:(t + 1) * P, :])

                # 2. per-chunk maxes
                cmax = small.tile([P, NCHUNK], mybir.dt.float32, tag="cmax")
                xv = x[:].rearrange("p (a b) -> p a b", b=CH)
                nc.vector.tensor_reduce(
                    cmax[:, :NCHUNK // 2], xv[:, :NCHUNK // 2, :],
                    axis=mybir.AxisListType.X, op=mybir.AluOpType.max,
                )
                nc.vector.tensor_reduce(
                    cmax[:, NCHUNK // 2:], xv[:, NCHUNK // 2:, :],
                    axis=mybir.AxisListType.X, op=mybir.AluOpType.max,
                )

                # 3. winning chunk per row (first occurrence of the max)
                m8 = small.tile([P, 8], mybir.dt.float32, tag="m8")
                nc.vector.max(out=m8[:], in_=cmax[:])
                ci8 = small.tile([P, 8], mybir.dt.uint32, tag="ci8")
                nc.vector.max_index(out=ci8[:], in_max=m8[:], in_values=cmax[:])
                cif = small.tile([P, 1], mybir.dt.float32, tag="cif")
                nc.vector.tensor_copy(cif[:], ci8[:, 0:1])

                # chunk-row id = (t*128 + p) * 25 + chunk
                rsf = small.tile([P, 1], mybir.dt.float32, tag="rsf")
                nc.vector.tensor_scalar(
                    rsf[:], cif[:], rowbase_f[:, 0:1], float(t * P * NCHUNK),
                    op0=fadd, op1=fadd,
                )
                rsi = small.tile([P, 1], mybir.dt.int32, tag="rsi")
                nc.vector.tensor_copy(rsi[:], rsf[:])

                # 4. gather winning chunk from HBM
                chunk = small.tile([P, CH], mybir.dt.float32, tag="chunk")
                nc.gpsimd.indirect_dma_start(
                    out=chunk[:], out_offset=None,
                    in_=pred_chunks,
                    in_offset=bass.IndirectOffsetOnAxis(ap=rsi[:, :1], axis=0),
                )

                # 5. position within chunk
                w8 = small.tile([P, 8], mybir.dt.float32, tag="w8")
                nc.vector.max(out=w8[:], in_=chunk[:])
                wi8 = small.tile([P, 8], mybir.dt.uint32, tag="wi8")
                nc.vector.max_index(out=wi8[:], in_max=w8[:], in_values=chunk[:])
                wif = small.tile([P, 1], mybir.dt.float32, tag="wif")
                nc.vector.tensor_copy(wif[:], wi8[:, 0:1])

                labf = small.tile([P, 1], mybir.dt.float32, tag="labf")
                nc.vector.tensor_scalar(
                    labf[:], cif[:], float(CH), None, op0=fmul)
                labf2 = small.tile([P, 1], mybir.dt.float32, tag="labf2")
                nc.vector.tensor_tensor(
                    out=labf2[:], in0=labf[:], in1=wif[:], op=fadd)
                labi = small.tile([P, 1], mybir.dt.int32, tag="labi")
                nc.vector.tensor_copy(labi[:], labf2[:])

                # 6. gather centers[label]
                csel = small.tile([P, FEAT_DIM], mybir.dt.float32, tag="csel")
                nc.gpsimd.indirect_dma_start(
                    out=csel[:], out_offset=None,
                    in_=centers.ap(),
                    in_offset=bass.IndirectOffsetOnAxis(ap=labi[:, :1], axis=0),
                )

                # 7. d = sum((f - c)^2)
                diff = small.tile([P, FEAT_DIM], mybir.dt.float32, tag="diff")
                nc.vector.tensor_sub(diff[:], f[:], csel[:])
                sq = small.tile([P, FEAT_DIM], mybir.dt.float32, tag="sq")
                nc.scalar.activation(
                    sq[:], diff[:], mybir.ActivationFunctionType.Square,
                    accum_out=acc[:, t:t + 1],
                )

            nc.sync.dma_start(out.ap()[:, :], acc[:])

    nc.compile()
    _CACHE[("nc", reps)] = nc
    return nc


def _build_null_nc():
    """Trivial NEFF (memset + tiny DMA out) to estimate launch overhead."""
    if "null" in _CACHE:
        return _CACHE["null"]

    from contextlib import ExitStack

    import concourse.tile as tile
    from concourse import bacc, mybir

    nc = bacc.Bacc(
        "TRN2", target_bir_lowering=False, debug=False, num_devices=N_CORES)
    predicts = nc.dram_tensor(
        "predicts", [ROWS_PER_CORE, NUM_CLASSES], mybir.dt.float32,
        kind="ExternalInput")
    features = nc.dram_tensor(
        "features", [ROWS_PER_CORE, FEAT_DIM], mybir.dt.float32,
        kind="ExternalInput")
    centers = nc.dram_tensor(
        "centers", [NUM_CLASSES, FEAT_DIM], mybir.dt.float32,
        kind="ExternalInput")
    out = nc.dram_tensor(
        "out", [P, N_TILES], mybir.dt.float32, kind="ExternalOutput")
    with tile.TileContext(nc) as tc:
        with ExitStack() as ctx:
            pool = ctx.enter_context(tc.tile_pool(name="p", bufs=1))
            acc = pool.tile([P, N_TILES], mybir.dt.float32)
            nc.vector.memset(acc[:], 0.0)
            nc.sync.dma_start(out.ap()[:, :], acc[:])
    nc.compile()
    _CACHE["null"] = nc
    return nc


def kernel(features, predicts, centers):
    from concourse.bass_utils import run_bass_kernel_spmd

    nc = _build_nc()

    feats = np.ascontiguousarray(
        np.asarray(features, dtype=np.float32).reshape(N_ROWS, FEAT_DIM))
    preds = np.ascontiguousarray(
        np.asarray(predicts, dtype=np.float32).reshape(N_ROWS, NUM_CLASSES))
    cents = np.ascontiguousarray(np.asarray(centers, dtype=np.float32))

    in_maps = []
    for m in range(N_CORES):
        s = slice(m * ROWS_PER_CORE, (m + 1) * ROWS_PER_CORE)
        in_maps.append({
            "predicts": np.ascontiguousarray(preds[s]),
            "features": np.ascontiguousarray(feats[s]),
            "centers": cents,
        })

    res = run_bass_kernel_spmd(nc, in_maps, core_ids=list(range(N_CORES)))

    d = np.concatenate([r["out"].reshape(-1) for r in res.results])
    d = np.clip(d.astype(np.float64), 1e-12, 1e12)
    total = d.sum() + (N_ROWS * NUM_CLASSES - N_ROWS) * 1e-12
    return np.asarray(total / N_ROWS, dtype=np.float32)
